# revision 18
# baseline (speedup 1.0000x reference)
"""Biaffine scorer kernel for Trainium2 (Bass/Tile), data-parallel over batch
across 8 NeuronCores — bf16 pipeline, streaming schedule (v4).

v4 reworks v3's schedule around the DMA roofline (~9MB of HBM traffic/core):

  - cmat input halved: only rows 0..127 are loaded. Rows 128..255 satisfy
    cmat[x+128, o, y] = cmat[x, o, y-128] for y>=128 and = 0 for y<128 once
    cls_b is folded into the ut matrix (ones x ones position) so that
    wproj[0] = 0 (width_table padding row). sb_c1 is rebuilt on-device with
    one memset + two strided DVE copies.
  - load order: weights -> pair-0 state -> cmat -> pair-1 state, so the PE
    starts projecting at ~3us and never waits on cmat for evacuation.
  - pair-1 projection matmuls are interleaved into pair-0's finals c-groups,
    removing the serial proj block between pairs; the PE stays warm and
    finishes ~7us earlier.
  - output pieces go out on the second HWDGE ring (nc.scalar) so they don't
    queue behind input loads on the SP ring; pieces stream per column-range
    as soon as each tile's chunks evacuate.
"""

import numpy as np
import ml_dtypes

import concourse.bass as bass
import concourse.bacc as bacc
import concourse.tile as tile
from concourse import mybir
from concourse.bass_utils import run_bass_kernel_spmd

# problem shape (hardcoded per harness contract)
B, S, H = 32, 255, 1024
BS, WD, O = 120, 20, 10
SP = 256            # padded S
SP2 = 2 * SP        # paired moving dim
NW = SP * O         # 2560
KT = H // 128       # 8
NCORES = 8
BPC = B // NCORES   # 4 batch items per core
NP = BPC // 2       # 2 pairs per core
BSE = BS + 1        # 121
UTW = O * BSE + 6   # 1216 (pad to keep 4B-aligned rows)

F32 = mybir.dt.float32
F16 = mybir.dt.bfloat16
BF16NP = ml_dtypes.bfloat16

_CACHE: dict = {}


def _emit(tc, d):
    """Emit the per-core program. d: dict of DRAM APs."""
    from contextlib import ExitStack

    nc = tc.nc
    AF = mybir.ActivationFunctionType

    with ExitStack() as ctx:
        const = ctx.enter_context(tc.tile_pool(name="const", bufs=1))
        st_pool = ctx.enter_context(tc.tile_pool(name="st", bufs=NP * 4))
        ht_pool = ctx.enter_context(tc.tile_pool(name="ht", bufs=4))
        tut_pool = ctx.enter_context(tc.tile_pool(name="tut", bufs=2))
        out_pool = ctx.enter_context(tc.tile_pool(name="outp", bufs=8))
        pp_ht = ctx.enter_context(tc.tile_pool(name="pp_ht", bufs=2, space="PSUM"))
        pp_u = ctx.enter_context(tc.tile_pool(name="pp_u", bufs=2, space="PSUM"))
        pp_s = ctx.enter_context(tc.tile_pool(name="pp_s", bufs=4, space="PSUM"))

        # ---- persistent constants + stateT, in consumption order on the SP
        # HWDGE ring (each dma_start costs ~0.6us of ring issue time, so
        # weights ride in ONE packed transfer): weights feed pair-0 proj
        # immediately; cmat lands right when the first finals evacuate;
        # pair-1 state streams last ----
        WPK = 2 * KT * BSE + 4 + UTW   # 3156
        WA = KT * BSE + 4              # tw | bias  (first transfer)
        sb_wp = const.tile([128, WPK], F16)
        sb_tw = sb_wp[:, 0:KT * BSE]
        sb_bias = sb_wp[0:BSE, KT * BSE:WA].bitcast(F32)
        sb_hw = sb_wp[:, WA:WA + KT * BSE]
        sb_ut = sb_wp[0:BSE, WA + KT * BSE:WPK]
        sb_cA = const.tile([128, 1024], F16)   # cmat rows 0:128, cols 0:1024
        sb_cB = const.tile([128, 1536], F16)   # cmat rows 0:128, cols 1024:2560
        sb_c1 = const.tile([128, NW], F16)     # derived rows 128:256
        stq = [
            st_pool.tile([128, 2 * SP2], F16, name=f"stq0_{q}", tag="stq")
            for q in range(4)
        ] + [
            st_pool.tile([128, 4 * SP2], F16, name=f"stq1_{h}", tag="stq1")
            for h in range(2)
        ]
        loads = [
            (sb_wp[:, 0:WA], d["wpack"][:, 0:WA]),
            (stq[0][:], d["st0"][0]),
            (stq[1][:], d["st0"][1]),
            (sb_wp[:, WA:WPK], d["wpack"][:, WA:WPK]),
            (stq[2][:], d["st0"][2]),
            (stq[3][:], d["st0"][3]),
            (sb_cA[:], d["cmat"][:, 0:1024]),
            (sb_cB[:], d["cmat"][:, 1024:2560]),
            (stq[4][:], d["st1"][0]),
            (stq[5][:], d["st1"][1]),
        ]
        for dst, src_ in loads:
            nc.sync.dma_start(dst, src_)

        # ---- PE warm-up: ~3.4us of continuous busy to leave the 1.2GHz
        # p-state; depends only on a DVE memset so it runs under the DMA head
        scratch = const.tile([128, 512], F16)
        nc.vector.memset(scratch[:], 0)
        for i in range(8):
            ps_d = pp_ht.tile([128, 512], F32, name=f"dmy_{i}", tag="ps")
            nc.tensor.matmul(
                ps_d[:], lhsT=scratch[:, 0:128], rhs=scratch[:],
                start=True, stop=True,
            )

        # ---- derive cmat rows 128:256 in SBUF: zero the y<128 halves, then
        # shift-copy the y>=128 halves from rows 0:128 (see module docstring)
        c1v = sb_c1[:].rearrange("p (o y) -> p o y", o=O)
        cAv = sb_cA[:].rearrange("p (o y) -> p o y", o=4)
        cBv = sb_cB[:].rearrange("p (o y) -> p o y", o=6)
        nc.gpsimd.memset(c1v[:, :, 0:128], 0)
        nc.vector.tensor_scalar_add(c1v[:, 0:4, 128:256], cAv[:, :, 0:128], 0.0)
        nc.vector.tensor_scalar_add(c1v[:, 4:10, 128:256], cBv[:, :, 0:128], 0.0)

        # ---- projection helpers (lazy psum per (pair, which)) ----
        proj_ps: dict = {}
        proj_hv: dict = {}

        def proj_mm(p, which, kt):
            key = (p, which)
            if key not in proj_ps:
                proj_ps[key] = pp_ht.tile(
                    [BSE, SP2], F32, name=f"ps_p{p}_{which}", tag="ps"
                )
            w = sb_tw if which else sb_hw
            if p == 0:
                st = stq[kt // 2]
                rhs = st[:, (kt % 2) * SP2:(kt % 2 + 1) * SP2]
            else:
                st = stq[4 + kt // 4]
                rhs = st[:, (kt % 4) * SP2:(kt % 4 + 1) * SP2]
            nc.tensor.matmul(
                proj_ps[key][:],
                lhsT=w[:, kt * BSE:(kt + 1) * BSE],
                rhs=rhs,
                start=(kt == 0),
                stop=(kt == KT - 1),
            )

        def proj_leaky(p, which):
            bcol = 1 if which else 0
            hv = ht_pool.tile([BSE, SP2], F16, name=f"ht_p{p}_{which}", tag="hv")
            # leaky(psum + bias); row 120: weights col is 0, bias 1 -> 1.0
            nc.scalar.activation(
                hv[:], proj_ps[(p, which)][:], AF.Lrelu,
                bias=sb_bias[:, bcol:bcol + 1], scale=1.0, alpha=0.01,
            )
            proj_hv[(p, which)] = hv

        def cmat_ap(xt, cc):
            if xt == 1:
                return sb_c1[:, cc * 512:(cc + 1) * 512]
            if cc < 2:
                return sb_cA[:, cc * 512:(cc + 1) * 512]
            return sb_cB[:, (cc - 2) * 512:(cc - 1) * 512]

        kevac = 0

        def emit_pair(p, interleave):
            """tut/finals c-group pipeline for pair p; `interleave` is a list
            of callables inserted one per finals tile (pair-1 proj MMs)."""
            nonlocal kevac
            t1T = proj_hv[(p, 1)]
            h1T = proj_hv[(p, 0)]
            tut = tut_pool.tile([BSE, O * SP2], F16, name=f"tut_{p}", tag="tut")
            outs = [
                out_pool.tile([128, NW], F16, name=f"sb_out_p{p}_{i}", tag="sb_out")
                for i in range(4)
            ]
            ii = 0  # interleave cursor
            for c in range(6):
                if c < 5:
                    for half in range(2):
                        o = 2 * c + half
                        ps_u = pp_u.tile(
                            [BSE, SP2], F32, name=f"ps_u_{p}_{o}", tag="ps_u"
                        )
                        nc.tensor.matmul(
                            ps_u[:],
                            lhsT=sb_ut[:, o * BSE:(o + 1) * BSE],
                            rhs=t1T[:],
                            start=True,
                            stop=True,
                        )
                        tdst = tut[:, o * SP2:(o + 1) * SP2]
                        nc.scalar.activation(tdst, ps_u[:], AF.Copy)
                if c >= 1:
                    cc = c - 1
                    for i in range(4):
                        bb, xt = i // 2, i % 2
                        sb_out = outs[i]
                        lo = bb * SP + xt * 128
                        ps_s = pp_s.tile(
                            [128, 512], F32, name=f"ps_s_{p}_{cc}_{i}", tag="ps_s"
                        )
                        for half in range(2):
                            o = 2 * cc + half
                            nc.tensor.matmul(
                                ps_s[:, half * 256:(half + 1) * 256],
                                lhsT=h1T[:, lo:lo + 128],
                                rhs=tut[:, o * SP2 + bb * SP:o * SP2 + bb * SP + SP],
                                start=True,
                                stop=True,
                            )
                        if c >= 2 and ii < len(interleave):
                            interleave[ii]()
                            ii += 1
                        oc = sb_out[:, cc * 512:(cc + 1) * 512]
                        co = cmat_ap(xt, cc)
                        if kevac % 6 == 5:
                            nc.scalar.activation(oc, ps_s[:], AF.Copy)
                            nc.gpsimd.tensor_add(oc, oc, co)
                        else:
                            nc.vector.tensor_add(oc, ps_s[:], co)
                        kevac += 1
                        # stream the output in 2 column pieces per tile; the
                        # first rides the SP HWDGE ring, the second goes out
                        # via SWDGE so neither ring's ~0.6us-per-DMA issue
                        # cadence serializes all 16 pieces
                        pieces = {2: (0, 1536), 4: (1536, NW)}
                        if cc in pieces:
                            a, bnd = pieces[cc]
                            eng = nc.sync if cc == 2 else nc.gpsimd
                            eng.dma_start(
                                d["out"][2 * p + bb, xt * 128:(xt + 1) * 128, a:bnd],
                                sb_out[:, a:bnd],
                            )

        # ---- pair 0: projections up front (gated by state quarters) ----
        for kt in range(KT):
            proj_mm(0, 1, kt)
        proj_leaky(0, 1)
        for kt in range(KT):
            proj_mm(0, 0, kt)
        proj_leaky(0, 0)

        # pair-1 proj interleaved into pair-0's finals groups c=2..5
        inter = []
        for kt in range(KT):
            def f(kt=kt):
                proj_mm(1, 1, kt)
                if kt == KT - 1:
                    proj_leaky(1, 1)
            inter.append(f)
        for kt in range(KT):
            def g(kt=kt):
                proj_mm(1, 0, kt)
                if kt == KT - 1:
                    proj_leaky(1, 0)
            inter.append(g)

        emit_pair(0, inter)
        emit_pair(1, [])


def build_nc():
    if "nc" in _CACHE:
        return _CACHE["nc"]
    nc = bacc.Bacc(
        "TRN2", target_bir_lowering=False, debug=False, num_devices=NCORES
    )
    d = {}
    WPK = 2 * KT * BSE + 4 + UTW
    d["st0"] = nc.dram_tensor(
        "st0", [4, 128, 1024], F16, kind="ExternalInput"
    ).ap()
    d["st1"] = nc.dram_tensor(
        "st1", [2, 128, 2048], F16, kind="ExternalInput"
    ).ap()
    d["wpack"] = nc.dram_tensor("wpack", [128, WPK], F16, kind="ExternalInput").ap()
    d["cmat"] = nc.dram_tensor("cmat", [128, NW], F16, kind="ExternalInput").ap()
    d["out"] = nc.dram_tensor("out", [BPC, SP, NW], F16, kind="ExternalOutput").ap()

    with tile.TileContext(nc) as tc:
        _emit(tc, d)
    nc.compile()
    _CACHE["nc"] = nc
    return nc


def prep_inputs(inputs):
    """Host-side packing + fp32->bf16 conversion. Returns dict of np arrays
    shared across cores (stateT is full-batch; shard before dispatch)."""
    state = np.asarray(inputs["state"], np.float32)
    head_w = np.asarray(inputs["head_w"], np.float32)
    head_b = np.asarray(inputs["head_b"], np.float32)
    tail_w = np.asarray(inputs["tail_w"], np.float32)
    tail_b = np.asarray(inputs["tail_b"], np.float32)
    U = np.asarray(inputs["U"], np.float32)
    width_table = np.asarray(inputs["width_table"], np.float32)
    cls_w = np.asarray(inputs["cls_w"], np.float32)
    cls_b = np.asarray(inputs["cls_b"], np.float32)

    # stateT paired pack: [B/2, 128, (kt, b01, y)], y zero-padded to 256
    stateT = np.zeros((B, H, SP), np.float32)
    stateT[:, :, :S] = state.transpose(0, 2, 1)
    stateT = stateT.reshape(B // 2, 2, KT, 128, SP).transpose(0, 3, 2, 1, 4)
    stateT = np.ascontiguousarray(
        stateT.reshape(B // 2, 128, KT * SP2).astype(BF16NP)
    )

    hw_sb = np.zeros((128, KT, BSE), np.float32)
    hw_sb[:, :, :BS] = head_w.reshape(KT, 128, BS).transpose(1, 0, 2)
    hw_sb = hw_sb.reshape(128, KT * BSE).astype(BF16NP)
    tw_sb = np.zeros((128, KT, BSE), np.float32)
    tw_sb[:, :, :BS] = tail_w.reshape(KT, 128, BS).transpose(1, 0, 2)
    tw_sb = tw_sb.reshape(128, KT * BSE).astype(BF16NP)

    # ut blocks: [j, o, i] = U[o,i,j]; col 120 = Wt_ext; row 120 += Wh_ext;
    # cls_b folded into [120, o, 120] (multiplied by ones x ones)
    ut = np.zeros((BSE, UTW), np.float32)
    blocks = ut[:, :O * BSE].reshape(BSE, O, BSE)
    blocks[:BS, :, :BS] = U.transpose(2, 0, 1)
    blocks[:, :, BS] = cls_w[:, BS + 1:2 * (BS + 1)].T
    blocks[BS, :, :] += cls_w[:, :BSE]
    blocks[BS, :, BS] += cls_b
    ut = np.ascontiguousarray(ut.astype(BF16NP))

    bias2 = np.zeros((BSE, 2), np.float32)
    bias2[:BS, 0] = head_b
    bias2[BS, 0] = 1.0
    bias2[:BS, 1] = tail_b
    bias2[BS, 1] = 1.0

    # cmat[x, o*256+y] = wproj[pos(x,y), o] for x<128 only; wproj excludes
    # cls_b so wproj[0] = 0 and rows 128:256 are kernel-derived
    pos = np.arange(S)[None, :] - np.arange(S)[:, None] + 1
    pos = pos * (pos > 0)
    posP = np.zeros((SP, SP), np.int64)
    posP[:S, :S] = pos
    wproj = width_table @ cls_w[:, 2 * (BS + 1):].T        # [256, 10], row 0 = 0
    cmat = wproj[posP[:128]]                               # [128, y, o]
    cmat = np.ascontiguousarray(
        cmat.transpose(0, 2, 1).reshape(128, NW).astype(BF16NP)
    )

    # packed weights: tw | bias2-as-bf16-bits | hw | ut (rows 0:121 used)
    WPK = 2 * KT * BSE + 4 + UTW
    WA = KT * BSE + 4
    wpack = np.zeros((128, WPK), BF16NP)
    wpack[:, 0:KT * BSE] = tw_sb
    wpack[:BSE, KT * BSE:WA] = bias2.view(BF16NP)
    wpack[:, WA:WA + KT * BSE] = hw_sb
    wpack[:BSE, WA + KT * BSE:WPK] = ut
    wpack = np.ascontiguousarray(wpack)

    # per-core split into contiguous quarter (pair 0) / half (pair 1) blocks
    st0 = np.ascontiguousarray(
        stateT.reshape(B // 2, 128, 4, 1024).transpose(0, 2, 1, 3)
    )  # [B/2, 4, 128, 1024]
    st1 = np.ascontiguousarray(
        stateT.reshape(B // 2, 128, 2, 2048).transpose(0, 2, 1, 3)
    )  # [B/2, 2, 128, 2048]

    return {
        "st0": st0,
        "st1": st1,
        "wpack": wpack,
        "cmat": cmat,
    }


def run(inputs, trace=False, trace_kwargs=None):
    nc = build_nc()
    full = prep_inputs(inputs)
    shared = {k: v for k, v in full.items() if k not in ("st0", "st1")}
    in_maps = []
    for c in range(NCORES):
        m = dict(shared)
        m["st0"] = np.ascontiguousarray(full["st0"][c * NP])
        m["st1"] = np.ascontiguousarray(full["st1"][c * NP + 1])
        in_maps.append(m)
    res = run_bass_kernel_spmd(
        nc,
        in_maps,
        core_ids=list(range(NCORES)),
        trace=trace,
        **(trace_kwargs or {}),
    )
    out = np.concatenate([r["out"] for r in res.results], axis=0)
    # [B, x(256), (o,y)] bf16 -> [B, x, y, o] fp32, trim padding
    out = out.astype(np.float32).reshape(B, SP, O, SP)
    out = np.ascontiguousarray(out.transpose(0, 1, 3, 2)[:, :S, :S, :])
    return out, res


def kernel(**inputs):
    out, _ = run(inputs, trace=False)
    return out


if __name__ == "__main__":
    build_nc()
    print("build ok")
